# revision 11
# baseline (speedup 1.0000x reference)
"""Trainium2 Bass kernel for nn_AttentionS2 (spherical self-attention).

Module: y = p_w @ softmax_k(q k^T / sqrt(hd) + log_quad_w[k]) v + p_b
with q/k/v = 1x1-conv projections of the same input (self-attention),
B=1, C=512, H=W=64 (4096 tokens), 8 heads, head_dim=64.

Sharding: one head per NeuronCore (8 cores). Each core:
  1. projects q_h, k_h (channel-major) and v_h^T (token-major) for its head
  2. computes S^T = k_h^T q_h in (key x query) orientation, 128-key tiles
  3. exp(scale*S^T + log_qw[key]) on the ACT engine (bias is per-partition
     = per-key in this orientation; 1024-wide spans amortize ACT overhead)
  4. accumulates [v^T | 1]^T @ P in PSUM -> numerator rows 0..63, denominator
     row 64 (the appended ones column), normalizes with a reciprocal +
     partition-broadcast multiply
  5. AllToAll reshards head-major outputs to token-major chunks
  6. applies the full output projection p_w on its 512-token slice
Host only slices weights per head and concatenates the 8 token slices.

Matmuls run in float32r (full-rate fp32 mode, ~1e-4 rel err); softmax
accumulation and PSUM stay fp32. Softmax skips max-subtraction: logits are
q.k/8 + log(quad weights) which is bounded well inside fp32 exp range.
"""

import contextlib
import sys
import types

import numpy as np

import concourse.bass as bass
import concourse.bacc as bacc
import concourse.tile as tile
from concourse import mybir
from concourse import bass_utils

# This container has no axon NTFF profile hook; shim the module so
# run_bass_kernel_spmd(trace=True) degrades gracefully instead of raising.
try:  # pragma: no cover
    import antenv.axon_hooks  # noqa: F401
except Exception:  # ModuleNotFoundError, or antenv missing entirely
    try:
        import antenv  # noqa: F401
    except Exception:
        antenv_mod = types.ModuleType("antenv")
        sys.modules["antenv"] = antenv_mod
    shim = types.ModuleType("antenv.axon_hooks")
    shim.get_axon_ntff_profile_hook = lambda: None
    sys.modules["antenv.axon_hooks"] = shim

F32 = mybir.dt.float32
F32R = mybir.dt.float32r
AF = mybir.ActivationFunctionType

C = 512          # channels
T = 4096         # tokens (H*W)
HD = 64          # head dim
NCORES = 8
NKT = T // 128   # 32 key tiles of 128
QC = 1024        # query chunk width for the attention inner loop
NQC = T // QC    # 4
CT = T // NCORES  # 512 tokens per core in the output projection
SCALE = 1.0 / float(np.sqrt(HD))

_CACHE = {}
_VARIANT = "full"   # "full" | "notail" (skip a2a + output projection; debug/timing)


def _emit_body(nc, tc, io, rep):
    """Emit one full forward pass. `io` holds the DRAM tensor handles.

    Emission order software-pipelines the attention inner loop: the S^T
    matmuls run two iterations ahead of exp/AV so the PE fills S(kt+2)
    while ACT computes exp(kt), breaking the exp->AV->S->exp serial chain.
    Projections are interleaved into the early attention iterations so the
    first exp can start ~10us in instead of after all projections.
    """
    x, wq, wk, wv, wp, ones, onesr, lqw, bq, bk, bv, pb, y = io
    with contextlib.ExitStack() as ctx:
        big = ctx.enter_context(tc.tile_pool(name=f"big{rep}", bufs=1))
        wts = ctx.enter_context(tc.tile_pool(name=f"wts{rep}", bufs=1))
        vtp = ctx.enter_context(tc.tile_pool(name=f"vtp{rep}", bufs=1))
        ptlp = ctx.enter_context(tc.tile_pool(name=f"ptl{rep}", bufs=4))
        sml = ctx.enter_context(tc.tile_pool(name=f"sml{rep}", bufs=2))
        drp = ctx.enter_context(tc.tile_pool(name=f"drp{rep}", bufs=1, space="DRAM"))

        ps_stack = contextlib.ExitStack()
        # one shared PSUM pool for projection + S staging (3 x 2 banks)
        # plus the AV accumulator (2 banks) = 8 banks exactly
        pss = ps_stack.enter_context(
            tc.tile_pool(name=f"pss{rep}", bufs=3, space="PSUM"))
        psa = ps_stack.enter_context(
            tc.tile_pool(name=f"psa{rep}", bufs=1, space="PSUM"))

        # ---- weight/const loads ---------------------------------------
        wq_sb = wts.tile([128, 4, HD], F32R, tag="wq")
        wk_sb = wts.tile([128, 4, HD], F32R, tag="wk")
        wv_sb = wts.tile([128, 4, HD], F32R, tag="wv")
        wp_sb = wts.tile([128, 4, C], F32R, tag="wp")
        for ci in range(4):
            cs = slice(128 * ci, 128 * (ci + 1))
            nc.sync.dma_start(out=wq_sb[:, ci, :], in_=wq[cs, :])
            nc.sync.dma_start(out=wk_sb[:, ci, :], in_=wk[cs, :])
            nc.sync.dma_start(out=wv_sb[:, ci, :], in_=wv[cs, :])
        lqw_sb = wts.tile([128, NKT], F32, tag="lqw")
        nc.sync.dma_start(out=lqw_sb, in_=lqw[:, :])
        bq_sb = wts.tile([HD, 1], F32, tag="bq")
        bk_sb = wts.tile([HD, 1], F32, tag="bk")
        bv_sb = wts.tile([HD, 1], F32, tag="bv")
        nc.sync.dma_start(out=bq_sb, in_=bq[:, :])
        nc.sync.dma_start(out=bk_sb, in_=bk[:, :])
        nc.sync.dma_start(out=bv_sb, in_=bv[:, :])
        pb_sb = wts.tile([128, 4], F32, tag="pb")
        nc.sync.dma_start(out=pb_sb, in_=pb[:, :])
        onesr_sb = wts.tile([1, HD], F32R, tag="onesr")
        nc.sync.dma_start(out=onesr_sb, in_=onesr[:, :])

        # ---- x loads, split by 1024-token groups so compute starts early
        x_sb = big.tile([128, 4, T], F32R, tag="x")
        def load_x_group(g):
            for ci in range(4):
                nc.sync.dma_start(
                    out=x_sb[:, ci, 1024 * g:1024 * (g + 1)],
                    in_=x[128 * ci:128 * (ci + 1), 1024 * g:1024 * (g + 1)])
        load_x_group(0)

        q_dup = big.tile([128, T], F32R, tag="qd")
        k_dup = big.tile([128, T], F32R, tag="kd")
        vt = []
        for t in range(NKT):
            vt_t = vtp.tile([128, HD + 1], F32R, tag=f"vt{t}")
            vt.append(vt_t)

        def emit_qk_chunk(w_sb, b_sb, dst, n):
            # channel-major projection of 512 tokens, duplicated to rows
            # 64:128 so S^T matmuls can row-pair two query subchunks
            ps = pss.tile([HD, 512], F32, tag="ss")
            for ci in range(4):
                nc.tensor.matmul(ps, w_sb[:, ci, :],
                                 x_sb[:, ci, 512 * n:512 * (n + 1)],
                                 start=(ci == 0), stop=(ci == 3))
            sl = slice(512 * n, 512 * (n + 1))
            nc.vector.tensor_scalar_add(out=dst[0:HD, sl], in0=ps, scalar1=b_sb)
            nc.sync.dma_start(out=dst[HD:128, sl], in_=dst[0:HD, sl])

        def emit_vt(t):
            # token-major v^T tile with appended ones column (denominator)
            ps = pss.tile([128, HD], F32, tag="ss")
            for ci in range(4):
                nc.tensor.matmul(ps, x_sb[:, ci, 128 * t:128 * (t + 1)],
                                 wv_sb[:, ci, :],
                                 start=(ci == 0), stop=(ci == 3))
            nc.vector.tensor_copy(out=vt[t][:, 0:HD], in_=ps)
            nc.sync.dma_start(out=vt[t][:, HD:HD + 1], in_=ones[:, :])

        # prologue: enough projections for the first attention iterations
        emit_qk_chunk(wq_sb, bq_sb, q_dup, 0)
        emit_qk_chunk(wq_sb, bq_sb, q_dup, 1)
        emit_qk_chunk(wk_sb, bk_sb, k_dup, 0)
        emit_vt(0)
        emit_vt(1)

        # ---- attention (flat software pipeline over (qc, kt)) ----------
        oh = big.tile([HD, T], F32R, tag="oh")
        snd = []
        rcv = []
        for j in range(NQC):
            snd_j = drp.tile([NCORES, HD, 128], F32R, tag=f"snd{j}",
                             name=f"snd{j}")
            rcv_j = drp.tile([NCORES, HD, 128], F32R, tag=f"rcv{j}",
                             name=f"rcv{j}")
            snd.append(snd_j)
            rcv.append(rcv_j)

        ss_tiles = {}

        def emit_s(qc, kt):
            ss = pss.tile([128, QC], F32, tag="ss")
            ss_tiles[(qc, kt)] = ss
            for sub in range(2):
                b0 = 64 * sub
                qoff = QC * qc + 512 * sub
                nc.tensor.matmul(ss[:, 512 * sub:512 * (sub + 1)],
                                 k_dup[b0:b0 + 64, 128 * kt:128 * (kt + 1)],
                                 q_dup[b0:b0 + 64, qoff:qoff + 512],
                                 start=True, stop=True)

        # interleaved projection work, keyed by global pipeline step.
        # During qc=0 we still owe: k chunks 1..7, vt 2..31, q chunks 2..7,
        # x token-groups 1..3, and the wp load for the final projection.
        prefetch = {}
        for i in range(1, 4):
            prefetch.setdefault(8 * i - 6, []).append(("xg", i))
        for n in range(1, 8):
            prefetch.setdefault(4 * n - 2, []).append(("k", n))
        for t in range(2, NKT):
            prefetch.setdefault(t - 1, []).append(("vt", t))
        for n in range(2, 8):
            prefetch.setdefault(3 * n + 4, []).append(("q", n))
        prefetch.setdefault(30, []).append(("wp",))

        steps = [(qc, kt) for qc in range(NQC) for kt in range(NKT)]
        av_tiles = {}
        emit_s(*steps[0])
        emit_s(*steps[1])
        for g, (qc, kt) in enumerate(steps):
            if qc == 0:
                for item in prefetch.get(g, ()):
                    if item[0] == "xg":
                        load_x_group(item[1])
                    elif item[0] == "k":
                        emit_qk_chunk(wk_sb, bk_sb, k_dup, item[1])
                    elif item[0] == "q":
                        emit_qk_chunk(wq_sb, bq_sb, q_dup, item[1])
                    elif item[0] == "vt":
                        emit_vt(item[1])
                    elif item[0] == "wp":
                        for ci in range(4):
                            nc.sync.dma_start(
                                out=wp_sb[:, ci, :],
                                in_=wp[128 * ci:128 * (ci + 1), :])
            if kt == 0:
                av_tiles[qc] = psa.tile([HD + 1, QC], F32, tag="av",
                                        name=f"av{qc}")
            av = av_tiles[qc]
            ss = ss_tiles.pop((qc, kt))
            pt = ptlp.tile([128, QC], F32R, tag="pt")
            nc.scalar.activation(out=pt, in_=ss, func=AF.Exp,
                                 scale=SCALE, bias=lqw_sb[:, kt:kt + 1])
            if g + 2 < len(steps):
                emit_s(*steps[g + 2])
            for sub in range(2):
                nc.tensor.matmul(av[:, 512 * sub:512 * (sub + 1)],
                                 vt[kt], pt[:, 512 * sub:512 * (sub + 1)],
                                 start=(kt == 0), stop=(kt == NKT - 1),
                                 skip_group_check=True)
            if kt == NKT - 1:
                # copy the finished accumulator out of PSUM immediately so
                # the AV bank frees for the next qc, then normalize:
                # rows 0..63 numerator, row 64 denominator
                av_sb = sml.tile([HD + 1, QC], F32, tag="avs")
                nc.vector.tensor_copy(out=av_sb, in_=av)
                rcp = sml.tile([1, QC], F32R, tag="rcp")
                with nc.allow_low_precision(
                        reason="1/den broadcast via f32r matmul; f32r keeps "
                               "~19 mantissa bits, fine for softmax scale"):
                    nc.vector.reciprocal(out=rcp, in_=av_sb[HD:HD + 1, :])
                qsl = slice(QC * qc, QC * (qc + 1))
                # broadcast 1/den across 64 partitions with a K=1 matmul
                rb = psa.tile([HD, QC], F32, tag="av", name=f"rb{qc}")
                for sub in range(2):
                    nc.tensor.matmul(rb[:, 512 * sub:512 * (sub + 1)],
                                     onesr_sb,
                                     rcp[:, 512 * sub:512 * (sub + 1)],
                                     start=True, stop=True)
                nc.vector.tensor_mul(out=oh[:, qsl], in0=av_sb[0:HD, :], in1=rb)
                nc.vector.tensor_scalar_add(out=oh[:, qsl], in0=oh[:, qsl],
                                            scalar1=bv_sb)
                # device qc holds quarter `qc` of every core's output chunk;
                # ship it now so the collective overlaps later qc compute
                for dest in range(NCORES):
                    so = QC * qc + 128 * dest
                    nc.sync.dma_start(out=snd[qc][dest, :, :],
                                      in_=oh[:, so:so + 128])
                nc.gpsimd.collective_compute(
                    "AllToAll", mybir.AluOpType.bypass,
                    replica_groups=[list(range(NCORES))],
                    ins=[snd[qc][:, :, :]], outs=[rcv[qc][:, :, :]])

        # ---- assemble token-major projection input, then output projection
        if _VARIANT == "notail":
            nc.gpsimd.dma_start(out=y[0:HD, :], in_=oh[:, 0:CT])
            ps_stack.close()
            return
        at = big.tile([128, 4, CT], F32R, tag="at")
        for j in range(NQC):
            rcv_flat = rcv[j][:, :, :].rearrange("a b c -> (a b) c")
            for ci in range(4):
                nc.sync.dma_start(
                    out=at[:, ci, 128 * j:128 * (j + 1)],
                    in_=rcv_flat[128 * ci:128 * (ci + 1), :])
        ps_stack.close()
        with tc.tile_pool(name=f"psy{rep}", bufs=2, space="PSUM") as psy:
            for m in range(4):
                ps = psy.tile([128, CT], F32, tag="yps")
                for ci in range(4):
                    nc.tensor.matmul(ps, wp_sb[:, ci, 128 * m:128 * (m + 1)],
                                     at[:, ci, :],
                                     start=(ci == 0), stop=(ci == 3))
                yo = sml.tile([128, CT], F32, tag="yo")
                nc.vector.tensor_scalar_add(out=yo, in0=ps,
                                            scalar1=pb_sb[:, m:m + 1])
                nc.sync.dma_start(out=y[128 * m:128 * (m + 1), :], in_=yo)


def _build(repeat=1):
    nc = bacc.Bacc("TRN2", target_bir_lowering=False, debug=False,
                   num_devices=NCORES)
    x = nc.dram_tensor("x", [C, T], F32R, kind="ExternalInput")
    wq = nc.dram_tensor("wq", [C, HD], F32R, kind="ExternalInput")
    wk = nc.dram_tensor("wk", [C, HD], F32R, kind="ExternalInput")
    wv = nc.dram_tensor("wv", [C, HD], F32R, kind="ExternalInput")
    wp = nc.dram_tensor("wp", [C, C], F32R, kind="ExternalInput")
    ones = nc.dram_tensor("ones", [128, 1], F32R, kind="ExternalInput")
    onesr = nc.dram_tensor("onesr", [1, HD], F32R, kind="ExternalInput")
    lqw = nc.dram_tensor("lqw", [128, NKT], F32, kind="ExternalInput")
    bq = nc.dram_tensor("bq", [HD, 1], F32, kind="ExternalInput")
    bk = nc.dram_tensor("bk", [HD, 1], F32, kind="ExternalInput")
    bv = nc.dram_tensor("bv", [HD, 1], F32, kind="ExternalInput")
    pb = nc.dram_tensor("pb", [128, 4], F32, kind="ExternalInput")
    y = nc.dram_tensor("y", [C, CT], F32, kind="ExternalOutput")
    io = (x, wq, wk, wv, wp, ones, onesr, lqw, bq, bk, bv, pb, y)

    with tile.TileContext(nc) as tc:
        for rep in range(repeat):
            _emit_body(nc, tc, io, rep)

    nc.finalize()
    return nc


def _get_nc(repeat=1):
    key = ("nc", repeat)
    if key not in _CACHE:
        _CACHE[key] = _build(repeat)
    return _CACHE[key]


def _token_perm():
    # device token 1024*j + 128*c + r  <->  original token 512*c + 128*j + r
    idx = np.empty(T, np.int64)
    for j in range(4):
        for c in range(NCORES):
            idx[1024 * j + 128 * c:1024 * j + 128 * (c + 1)] = \
                np.arange(512 * c + 128 * j, 512 * c + 128 * (j + 1))
    return idx


def _in_maps(query, q_w, q_b, k_w, k_b, v_w, v_b, p_w, p_b, log_quad_weights):
    idx = _token_perm()
    x = np.ascontiguousarray(
        np.asarray(query, np.float32).reshape(C, T)[:, idx])
    wp = np.ascontiguousarray(np.asarray(p_w, np.float32).T)
    pb = np.ascontiguousarray(np.asarray(p_b, np.float32).reshape(4, 128).T)
    lqw = np.ascontiguousarray(
        np.asarray(log_quad_weights, np.float32)[idx].reshape(NKT, 128).T)
    ones = np.ones((128, 1), np.float32)
    maps = []
    for h in range(NCORES):
        hs = slice(HD * h, HD * (h + 1))
        maps.append(dict(
            x=x,
            wq=np.ascontiguousarray(np.asarray(q_w, np.float32)[hs, :].T),
            wk=np.ascontiguousarray(np.asarray(k_w, np.float32)[hs, :].T),
            wv=np.ascontiguousarray(np.asarray(v_w, np.float32)[hs, :].T),
            wp=wp,
            ones=ones,
            onesr=np.ones((1, HD), np.float32),
            lqw=lqw,
            bq=np.ascontiguousarray(np.asarray(q_b, np.float32)[hs].reshape(HD, 1)),
            bk=np.ascontiguousarray(np.asarray(k_b, np.float32)[hs].reshape(HD, 1)),
            bv=np.ascontiguousarray(np.asarray(v_b, np.float32)[hs].reshape(HD, 1)),
            pb=pb,
        ))
    return maps


def _run(in_maps, repeat=1, **kw):
    nc = _get_nc(repeat)
    return bass_utils.run_bass_kernel_spmd(nc, in_maps, list(range(NCORES)), **kw)


def _assemble(results):
    # core c owns output tokens [CT*c, CT*(c+1))
    full = np.concatenate([results[c]["y"] for c in range(NCORES)], axis=1)
    return np.ascontiguousarray(full.reshape(1, C, 64, 64).astype(np.float32))


def kernel(**inputs):
    res = _run(_in_maps(**inputs))
    return _assemble(res.results)
